# revision 3
# baseline (speedup 1.0000x reference)
"""Multi-head causal attention (B=2, S=2048, D=1024, H=16) on 8 trn2 cores.

Sharding: core c -> (batch b = c//4, head-group g = c%4, 4 heads each).
Data-parallel over B, tensor-parallel over heads. Each core computes a
partial output projection [S, D]; the host sums the 4 partials per batch
and adds b_out.

v2 changes vs baseline:
  - bf16 end to end (x, weights, qkT, v_aug, valuesT, ex, out partials);
    PSUM stays f32. Halves DMA traffic and most DVE copy traffic.
  - causal mask applied by zeroing ex diagonal triangles on GpSimd
    (affine_select) instead of PE mask matmuls.
  - qk bias folded into the PSUM->SBUF copy (tensor_scalar_add with a
    per-partition bias column); v bias folded into the v_aug copy
    (tensor_tensor add with a pre-broadcast bias tile).
  - xT DMA'd in 512-column chunks, stage A runs nt-major so the first
    matmul starts after ~0.5MB of DMA instead of ~4MB. xt pool is
    double-buffered so half-1 DMA overlaps half-0 compute.

Device kernel per core (all matmuls bf16 -> f32 PSUM):
  A) qkT[f=512, s=2048] = (x @ Wqk)^T and v[s, f=256] = x @ Wv.
     qkT feature layout: [K(h0)|K(h1)] [Q(h0)|Q(h1)] [K(h2)|K(h3)] [Q(h2)|Q(h3)]
  B) per head h, per 512-wide query block qmb: causal flash attention in
     the scores-TRANSPOSED layout: sT[k,q] = K @ Q^T so that attn@V is
     lhsT=v_blk[s,hd+1] (ones col appended -> softmax denominators in
     row 64 of PSUM), rhs=expT[k,q]. No on-chip transposes anywhere.
  C) out_partial[s, 1024] = values^T.T @ W_out, DMA'd to DRAM (bf16).
"""
import math
import numpy as np
import ml_dtypes

import concourse.bass as bass
import concourse.mybir as mybir
import concourse.tile as tile
from concourse import bacc
from concourse.bass_utils import run_bass_kernel_spmd

N_CORES = 8
B, S, D = 2, 2048, 1024
H = 16                    # total heads
HL = 4                    # heads per core
HD = 64                   # head dim
FQK = 2 * HL * HD         # 512 local q+k features
FV = HL * HD              # 256 local v features
SCALE = 1.0 / math.sqrt(HD)

QMB = 512                 # query macro-block
KB = 128                  # key block
N_QMB = S // QMB          # 4
N_KB = S // KB            # 16

F32 = mybir.dt.float32
BF16 = mybir.dt.bfloat16
BF16_NP = ml_dtypes.bfloat16


def build_kernel(repeat: int = 1, pairw: int = 2, wave: int = 2,
                 sc_bufs: int = 3, av_bufs: int = 2, exp_bufs: int = 8,
                 lag: int = 2, qmb_order=(1, 2, 3, 0), a1_fill: bool = True):
    assert sc_bufs * pairw + av_bufs <= 8
    W = 512 * pairw
    nc = bacc.Bacc(
        "TRN2", target_bir_lowering=False, debug=False, num_devices=N_CORES
    )
    xT = nc.dram_tensor("xT", [D, S], BF16, kind="ExternalInput")
    wqk = nc.dram_tensor("wqk", [D, FQK], BF16, kind="ExternalInput")
    wv = nc.dram_tensor("wv", [D, FV], BF16, kind="ExternalInput")
    wo = nc.dram_tensor("wo", [FV, D], BF16, kind="ExternalInput")
    bqk = nc.dram_tensor("bqk", [FQK], F32, kind="ExternalInput")
    bv = nc.dram_tensor("bv", [2 * FV], F32, kind="ExternalInput")
    out = nc.dram_tensor("out", [S, D], BF16, kind="ExternalOutput")

    KT = D // 128  # 8 contraction tiles over D

    with tile.TileContext(nc) as tc:
        dma = nc.sync  # HWDGE: spreads transfers over HW queues
        with (
            tc.tile_pool(name="const", bufs=1) as const,
            tc.tile_pool(name="xt", bufs=2) as xtp,
            tc.tile_pool(name="big", bufs=1) as big,
            tc.tile_pool(name="exp", bufs=exp_bufs) as expp,
            tc.tile_pool(name="small", bufs=4) as small,
            tc.tile_pool(name="ob", bufs=3) as obp,
            tc.tile_pool(name="ps_sc", bufs=sc_bufs, space="PSUM") as ps_sc,
            tc.tile_pool(name="ps_av", bufs=av_bufs, space="PSUM") as ps_av,
        ):
            # ---- constants ----
            wqk_sb = const.tile([128, KT, FQK], BF16)
            wv_sb = const.tile([128, KT, FV], BF16)
            wo_sb = const.tile([128, FV // 128, D], BF16)
            dma.dma_start(
                out=wqk_sb, in_=wqk.rearrange("(kt p) f -> p kt f", p=128)
            )
            dma.dma_start(
                out=wv_sb, in_=wv.rearrange("(kt p) f -> p kt f", p=128)
            )
            dma.dma_start(
                out=wo_sb, in_=wo.rearrange("(dt p) f -> p dt f", p=128)
            )
            # qk bias as a per-partition column per f-tile: [128, ft]
            bqk_sb = const.tile([128, 4], F32)
            dma.dma_start(out=bqk_sb, in_=bqk.rearrange("(t p) -> p t", p=128))
            # v bias broadcast to all partitions: [128, 2*FV]
            bv_row = const.tile([1, 2 * FV], F32)
            dma.dma_start(out=bv_row, in_=bv.rearrange("(o f) -> o f", o=1))
            bv_bc = const.tile([128, 2 * FV], F32)
            nc.gpsimd.partition_broadcast(bv_bc, bv_row)

            # ---- persistent intermediates ----
            qkT = big.tile([128, 4, S], BF16)            # 4 f-tiles x S
            v_aug = big.tile([128, N_KB, HL, HD + 1], BF16)
            valuesT = big.tile([128, FV // 128, S], BF16)
            nc.vector.memset(v_aug[:, :, :, HD:HD + 1], 1.0)

            def body(_it):
                # ======== stage A: qkT and v_aug ========
                def load_xts(half):
                    s0 = half * (S // 2)
                    xts = []
                    for kt in range(KT):
                        xt_t = xtp.tile([128, S // 2], BF16, tag=f"xt{kt}")
                        for ch in range(2):
                            c0 = ch * 512
                            dma.dma_start(
                                out=xt_t[:, c0:c0 + 512],
                                in_=xT[kt * 128:(kt + 1) * 128,
                                       s0 + c0:s0 + c0 + 512],
                            )
                        xts.append(xt_t)
                    return xts

                def make_qk_item(xts, half, ft, nt):
                    s0 = half * (S // 2)

                    def emit():
                        c0 = nt * 512
                        ps = ps_sc.tile([128, W], F32, tag="sc")
                        for kt in range(KT):
                            nc.tensor.matmul(
                                ps[:, 0:512],
                                wqk_sb[:, kt, ft * 128:(ft + 1) * 128],
                                xts[kt][:, c0:c0 + 512],
                                start=(kt == 0),
                                stop=(kt == KT - 1),
                            )
                        nc.vector.tensor_scalar_add(
                            qkT[:, ft, s0 + c0:s0 + c0 + 512],
                            ps[:, 0:512],
                            bqk_sb[:, ft:ft + 1],
                        )
                    return emit

                def make_v_item(xts, half, stp):
                    def emit():
                        psv = ps_sc.tile([128, 512], F32, tag="sc")
                        for sub in range(2):
                            sti = stp * 2 + sub
                            c0 = sub * FV
                            for kt in range(KT):
                                nc.tensor.matmul(
                                    psv[:, c0:c0 + FV],
                                    xts[kt][:, sti * 128:(sti + 1) * 128],
                                    wv_sb[:, kt, :],
                                    start=(kt == 0),
                                    stop=(kt == KT - 1),
                                )
                        st0 = half * 8 + stp * 2
                        nc.vector.tensor_tensor(
                            v_aug[:, st0:st0 + 2, :, 0:HD],
                            psv.rearrange("s (t h c) -> s t h c", t=2, h=HL),
                            bv_bc.rearrange("s (t h c) -> s t h c", t=2, h=HL),
                            mybir.AluOpType.add,
                        )
                    return emit

                def a_items(xts, half):
                    # nt-major: items for columns 0:512 first, so compute
                    # starts after the first DMA chunk lands.
                    items = []
                    for nt in range(2):
                        for ft in range(4):
                            items.append(make_qk_item(xts, half, ft, nt))
                        items.append(make_v_item(xts, half, 2 * nt))
                        items.append(make_v_item(xts, half, 2 * nt + 1))
                    return items

                xts0 = load_xts(0)
                for it in a_items(xts0, 0):
                    it()
                xts1 = load_xts(1)
                a1 = a_items(xts1, 1)
                filler = []
                if a1_fill:
                    filler.extend(a1)
                else:
                    for it in a1:
                        it()

                # ======== stage B+C: per query macro-block ========
                def make_c_item(st):
                    def emit():
                        ob = obp.tile([128, 1024], BF16)
                        for nt in range(2):
                            ps = ps_sc.tile([128, W], F32, tag="sc")
                            for dt_ in range(FV // 128):
                                nc.tensor.matmul(
                                    ps[:, 0:512],
                                    valuesT[:, dt_, st * 128:(st + 1) * 128],
                                    wo_sb[:, dt_, nt * 512:(nt + 1) * 512],
                                    start=(dt_ == 0),
                                    stop=(dt_ == FV // 128 - 1),
                                )
                            nc.vector.tensor_copy(
                                ob[:, nt * 512:(nt + 1) * 512], ps[:, 0:512]
                            )
                        dma.dma_start(
                            out=out[st * 128:(st + 1) * 128, :], in_=ob
                        )
                    return emit

                n_qmb_done = 0
                for qmb in qmb_order:
                    n_qmb_done += 1
                    if n_qmb_done == 3:
                        while filler:
                            filler.pop(0)()
                    q0 = qmb * QMB
                    nkb = 4 * qmb + 4
                    nblk = (nkb + pairw - 1) // pairw
                    for w0 in range(0, HL, wave):
                        whs = list(range(w0, w0 + wave))
                        avs = {
                            h_: ps_av.tile([65, QMB], F32, tag="av",
                                           name=f"av{h_}")
                            for h_ in whs
                        }
                        avq = []

                        def emit_av(item):
                            h, mms = item
                            for kb, col0, avw, ex_t in mms:
                                nc.tensor.matmul(
                                    avs[h][0:65, col0:col0 + avw],
                                    v_aug[:, kb, h, :],
                                    ex_t,
                                    start=(kb == 0),
                                    stop=(kb == nkb - 1),
                                )

                        for blk in range(nblk):
                            kb0 = blk * pairw
                            scs = {}
                            # row-packed: both heads' score MMs emitted
                            # back-to-back; lhsT base partitions 0/64 ->
                            # concurrent row-group execution on the PE.
                            for h in whs:
                                scs[h] = ps_sc.tile(
                                    [128, W], F32, tag="sc", name=f"sc{h}"
                                )
                            for sub in range(pairw):
                                kb = kb0 + sub
                                j = kb - 4 * qmb
                                col0 = 128 * j if j >= 0 else 0
                                cb = sub * 512 + col0
                                scw = 512 - col0
                                for h in whs:
                                    tk = 2 * (h // 2)
                                    pk = 64 * (h % 2)
                                    nc.tensor.matmul(
                                        scs[h][:, cb:cb + scw],
                                        qkT[pk:pk + 64, tk,
                                            kb * KB:(kb + 1) * KB],
                                        qkT[pk:pk + 64, tk + 1,
                                            q0 + col0:q0 + col0 + scw],
                                        start=True,
                                        stop=True,
                                        skip_group_check=True,
                                    )
                            for h in whs:
                                sc = scs[h]
                                ex = expp.tile([128, W], BF16)
                                diag = kb0 + pairw - 1 >= 4 * qmb
                                if diag:
                                    for sub in range(pairw):
                                        j = kb0 + sub - 4 * qmb
                                        col0 = 128 * j if j >= 0 else 0
                                        cb = sub * 512 + col0
                                        nc.scalar.activation(
                                            out=ex[:, cb:sub * 512 + 512],
                                            in_=sc[:, cb:sub * 512 + 512],
                                            func=(mybir
                                                  .ActivationFunctionType.Exp),
                                            scale=SCALE,
                                        )
                                        if j >= 0:
                                            # zero ex where k > q in the
                                            # 128x128 diagonal chunk
                                            nc.gpsimd.affine_select(
                                                out=ex[:, cb:cb + 128],
                                                in_=ex[:, cb:cb + 128],
                                                compare_op=(mybir.AluOpType
                                                            .is_ge),
                                                fill=0.0,
                                                base=0,
                                                pattern=[[1, 128]],
                                                channel_multiplier=-1,
                                            )
                                else:
                                    nc.scalar.activation(
                                        out=ex,
                                        in_=sc,
                                        func=mybir.ActivationFunctionType.Exp,
                                        scale=SCALE,
                                    )
                                mms = []
                                for sub in range(pairw):
                                    kb = kb0 + sub
                                    j = kb - 4 * qmb
                                    col0 = 128 * j if j >= 0 else 0
                                    mms.append((
                                        kb, col0, QMB - col0,
                                        ex[:, sub * 512 + col0:
                                            sub * 512 + QMB],
                                    ))
                                avq.append((h, mms))
                            while len(avq) > wave * lag:
                                emit_av(avq.pop(0))
                            if filler:
                                filler.pop(0)()
                        while avq:
                            emit_av(avq.pop(0))

                        # normalize: values = av[0:64] / av[64]
                        for h in whs:
                            av = avs[h]
                            rec = small.tile([1, QMB], BF16, tag="rec")
                            with nc.allow_low_precision(
                                reason="softmax denom feeds bf16 matmul"
                            ):
                                nc.vector.reciprocal(rec, av[64:65, :])
                            rb = small.tile([64, QMB], BF16, tag="rb")
                            nc.gpsimd.partition_broadcast(rb, rec)
                            dt_ = h // 2
                            pr = 64 * (h % 2)
                            nc.vector.tensor_mul(
                                valuesT[pr:pr + 64, dt_, q0:q0 + QMB],
                                av[0:64, :],
                                rb,
                            )
                    # ---- queue stage C for this qmb ----
                    for sti in range(QMB // 128):
                        filler.append(make_c_item(qmb * 4 + sti))
                while filler:
                    filler.pop(0)()

            if repeat == 1:
                body(0)
            else:
                with tc.For_i(
                    0, repeat, 1,
                    hint_engines=(mybir.EngineType.PE,),
                ) as it:
                    body(it)
    nc.compile()
    return nc


def make_in_maps(x, W_qkv, b_qkv, W_out, b_out):
    """Host-side sharding: per-core input dict."""
    x = np.asarray(x, dtype=np.float32)
    W_qkv = np.asarray(W_qkv, dtype=np.float32)
    b_qkv = np.asarray(b_qkv, dtype=np.float32)
    W_out = np.asarray(W_out, dtype=np.float32)
    in_maps = []
    xT_by_b = [
        np.ascontiguousarray(x[b_].T).astype(BF16_NP) for b_ in range(B)
    ]
    for c in range(N_CORES):
        b_ = c // 4
        g = c % 4
        heads = [4 * g + i for i in range(HL)]
        # feature order: K(h0),K(h1),Q(h0),Q(h1),K(h2),K(h3),Q(h2),Q(h3)
        qk_cols = []
        for pair in range(2):
            h0, h1 = heads[2 * pair], heads[2 * pair + 1]
            for h_ in (h0, h1):
                base = h_ * 3 * HD + 1 * HD  # K
                qk_cols.extend(range(base, base + HD))
            for h_ in (h0, h1):
                base = h_ * 3 * HD + 0 * HD  # Q
                qk_cols.extend(range(base, base + HD))
        v_cols = []
        for h_ in heads:
            base = h_ * 3 * HD + 2 * HD  # V
            v_cols.extend(range(base, base + HD))
        qk_cols = np.array(qk_cols)
        v_cols = np.array(v_cols)
        bv_local = np.ascontiguousarray(b_qkv[v_cols], dtype=np.float32)
        in_maps.append({
            "xT": xT_by_b[b_],
            "wqk": np.ascontiguousarray(W_qkv[:, qk_cols]).astype(BF16_NP),
            "wv": np.ascontiguousarray(W_qkv[:, v_cols]).astype(BF16_NP),
            "wo": np.ascontiguousarray(
                W_out[g * FV:(g + 1) * FV, :]
            ).astype(BF16_NP),
            "bqk": np.ascontiguousarray(b_qkv[qk_cols], dtype=np.float32),
            "bv": np.tile(bv_local, 2),
        })
    return in_maps


_NC_CACHE = {}


def get_nc(repeat: int = 1):
    if repeat not in _NC_CACHE:
        _NC_CACHE[repeat] = build_kernel(repeat)
    return _NC_CACHE[repeat]


def kernel(x, W_qkv, b_qkv, W_out, b_out):
    in_maps = make_in_maps(x, W_qkv, b_qkv, W_out, b_out)
    nc = get_nc(1)
    res = run_bass_kernel_spmd(nc, in_maps, list(range(N_CORES)))
    b_out = np.asarray(b_out, dtype=np.float32)
    out = np.zeros((B, S, D), dtype=np.float32)
    for b_ in range(B):
        acc = np.zeros((S, D), dtype=np.float32)
        for g in range(4):
            acc += res.results[4 * b_ + g]["out"].astype(np.float32)
        out[b_] = acc + b_out[None, :]
    return out


# revision 16
# speedup vs baseline: 1.1517x; 1.1517x over previous
"""Multi-head causal attention (B=2, S=2048, D=1024, H=16) on 8 trn2 cores.

Sharding: core c -> (batch b = c//4, head-group g = c%4, 4 heads each).
Data-parallel over B, tensor-parallel over heads. Each core computes a
partial output projection [S, D]; the host sums the 4 partials per batch
and adds b_out.

v2 changes vs baseline:
  - bf16 end to end (x, weights, qkT, v_aug, valuesT, ex, out partials);
    PSUM stays f32. Halves DMA traffic and most DVE copy traffic.
  - causal mask applied by zeroing ex diagonal triangles on GpSimd
    (affine_select) instead of PE mask matmuls.
  - qk bias folded into the PSUM->SBUF copy (tensor_scalar_add with a
    per-partition bias column); v bias folded into the v_aug copy
    (tensor_tensor add with a pre-broadcast bias tile).
  - xT DMA'd in 512-column chunks, stage A runs nt-major so the first
    matmul starts after ~0.5MB of DMA instead of ~4MB. xt pool is
    double-buffered so half-1 DMA overlaps half-0 compute.

Device kernel per core (all matmuls bf16 -> f32 PSUM):
  A) qkT[f=512, s=2048] = (x @ Wqk)^T and v[s, f=256] = x @ Wv.
     qkT feature layout: [K(h0)|K(h1)] [Q(h0)|Q(h1)] [K(h2)|K(h3)] [Q(h2)|Q(h3)]
  B) per head h, per 512-wide query block qmb: causal flash attention in
     the scores-TRANSPOSED layout: sT[k,q] = K @ Q^T so that attn@V is
     lhsT=v_blk[s,hd+1] (ones col appended -> softmax denominators in
     row 64 of PSUM), rhs=expT[k,q]. No on-chip transposes anywhere.
  C) out_partial[s, 1024] = values^T.T @ W_out, DMA'd to DRAM (bf16).
"""
import math
import numpy as np
import ml_dtypes

import concourse.bass as bass
import concourse.mybir as mybir
import concourse.tile as tile
from concourse import bacc
from concourse.bass_utils import run_bass_kernel_spmd

N_CORES = 8
B, S, D = 2, 2048, 1024
H = 16                    # total heads
HL = 4                    # heads per core
HD = 64                   # head dim
FQK = 2 * HL * HD         # 512 local q+k features
FV = HL * HD              # 256 local v features
SCALE = 1.0 / math.sqrt(HD)

QMB = 512                 # query macro-block
KB = 128                  # key block
N_QMB = S // QMB          # 4
N_KB = S // KB            # 16

F32 = mybir.dt.float32
BF16 = mybir.dt.bfloat16
BF16_NP = ml_dtypes.bfloat16


def build_kernel(repeat: int = 1, pairw: int = 2, wave: int = 2,
                 sc_bufs: int = 3, av_bufs: int = 2, exp_bufs: int = 8,
                 lag: int = 2, qmb_order=(1, 2, 3, 0), a1_fill: bool = True,
                 stages: str = "ABC", tiny_exp: bool = False,
                 quarters: bool = True):
    assert sc_bufs * pairw + av_bufs <= 8
    W = 512 * pairw
    nc = bacc.Bacc(
        "TRN2", target_bir_lowering=False, debug=False, num_devices=N_CORES
    )
    xT = nc.dram_tensor("xT", [D, S], BF16, kind="ExternalInput")
    wqk = nc.dram_tensor("wqk", [D, FQK], BF16, kind="ExternalInput")
    wv = nc.dram_tensor("wv", [D, FV], BF16, kind="ExternalInput")
    wo = nc.dram_tensor("wo", [FV, D], BF16, kind="ExternalInput")
    bqk = nc.dram_tensor("bqk", [FQK], F32, kind="ExternalInput")
    bv = nc.dram_tensor("bv", [2 * FV], F32, kind="ExternalInput")
    out = nc.dram_tensor("out", [S, D], BF16, kind="ExternalOutput")

    KT = D // 128  # 8 contraction tiles over D

    with tile.TileContext(nc) as tc:
        dma = nc.sync  # HWDGE: spreads transfers over HW queues
        with (
            tc.tile_pool(name="const", bufs=1) as const,
            tc.tile_pool(name="xt", bufs=2) as xtp,
            tc.tile_pool(name="big", bufs=1) as big,
            tc.tile_pool(name="exp", bufs=exp_bufs) as expp,
            tc.tile_pool(name="small", bufs=4) as small,
            tc.tile_pool(name="ob", bufs=3) as obp,
            tc.tile_pool(name="ps_sc", bufs=sc_bufs, space="PSUM") as ps_sc,
            tc.tile_pool(name="ps_av", bufs=av_bufs, space="PSUM") as ps_av,
        ):
            # ---- constants ----
            wqk_sb = const.tile([128, KT, FQK], BF16)
            wv_sb = const.tile([128, KT, FV], BF16)
            wo_sb = const.tile([128, FV // 128, D], BF16)
            dma.dma_start(
                out=wqk_sb, in_=wqk.rearrange("(kt p) f -> p kt f", p=128)
            )
            dma.dma_start(
                out=wv_sb, in_=wv.rearrange("(kt p) f -> p kt f", p=128)
            )
            dma.dma_start(
                out=wo_sb, in_=wo.rearrange("(dt p) f -> p dt f", p=128)
            )
            # qk bias as a per-partition column per f-tile: [128, ft]
            bqk_sb = const.tile([128, 4], F32)
            dma.dma_start(out=bqk_sb, in_=bqk.rearrange("(t p) -> p t", p=128))
            # v bias broadcast to all partitions: [128, 2*FV]
            bv_row = const.tile([1, 2 * FV], F32)
            dma.dma_start(out=bv_row, in_=bv.rearrange("(o f) -> o f", o=1))
            bv_bc = const.tile([128, 2 * FV], F32)
            nc.gpsimd.partition_broadcast(bv_bc, bv_row)

            # ---- persistent intermediates ----
            qkT = big.tile([128, 4, S], BF16)            # 4 f-tiles x S
            v_aug = big.tile([128, N_KB, HL, HD + 1], BF16)
            valuesT = big.tile([128, FV // 128, S], BF16)
            nc.vector.memset(v_aug[:, :, :, HD:HD + 1], 1.0)

            def body(_it):
                # ======== stage A: qkT and v_aug ========
                def load_xts(half):
                    s0 = half * (S // 2)
                    xts = []
                    for kt in range(KT):
                        xt_t = xtp.tile([128, S // 2], BF16, tag=f"xt{kt}")
                        for ch in range(2):
                            c0 = ch * 512
                            dma.dma_start(
                                out=xt_t[:, c0:c0 + 512],
                                in_=xT[kt * 128:(kt + 1) * 128,
                                       s0 + c0:s0 + c0 + 512],
                            )
                        xts.append(xt_t)
                    return xts

                def make_qk_item(xts, half, ft, nt):
                    s0 = half * (S // 2)

                    def emit():
                        c0 = nt * 512
                        ps = ps_sc.tile([128, W], F32, tag="sc")
                        for kt in range(KT):
                            nc.tensor.matmul(
                                ps[:, 0:512],
                                wqk_sb[:, kt, ft * 128:(ft + 1) * 128],
                                xts[kt][:, c0:c0 + 512],
                                start=(kt == 0),
                                stop=(kt == KT - 1),
                            )
                        nc.vector.tensor_scalar_add(
                            qkT[:, ft, s0 + c0:s0 + c0 + 512],
                            ps[:, 0:512],
                            bqk_sb[:, ft:ft + 1],
                        )
                    return emit

                def make_v_item(xts, half, stp):
                    def emit():
                        psv = ps_sc.tile([128, 512], F32, tag="sc")
                        for sub in range(2):
                            sti = stp * 2 + sub
                            c0 = sub * FV
                            for kt in range(KT):
                                nc.tensor.matmul(
                                    psv[:, c0:c0 + FV],
                                    xts[kt][:, sti * 128:(sti + 1) * 128],
                                    wv_sb[:, kt, :],
                                    start=(kt == 0),
                                    stop=(kt == KT - 1),
                                )
                        st0 = half * 8 + stp * 2
                        nc.vector.tensor_tensor(
                            v_aug[:, st0:st0 + 2, :, 0:HD],
                            psv.rearrange("s (t h c) -> s t h c", t=2, h=HL),
                            bv_bc.rearrange("s (t h c) -> s t h c", t=2, h=HL),
                            mybir.AluOpType.add,
                        )
                    return emit

                def a_items(xts, half):
                    # nt-major: items for columns 0:512 first, so compute
                    # starts after the first DMA chunk lands.
                    items = []
                    for nt in range(2):
                        for ft in range(4):
                            items.append(make_qk_item(xts, half, ft, nt))
                        items.append(make_v_item(xts, half, 2 * nt))
                        items.append(make_v_item(xts, half, 2 * nt + 1))
                    return items

                def quarter_items(xts, half, nt):
                    items = [
                        make_qk_item(xts, half, ft, nt) for ft in range(4)
                    ]
                    items.append(make_v_item(xts, half, 2 * nt))
                    items.append(make_v_item(xts, half, 2 * nt + 1))
                    return items

                # filler entries: (quarter_tag_or_None, fn)
                filler = []
                pending = [0, 0, 0, 0]  # un-emitted A items per quarter

                def pop_filler():
                    tag, fn = filler.pop(0)
                    fn()
                    if tag is not None:
                        pending[tag] -= 1

                def need_quarter(k):
                    # flush filler until A quarters <= k are all emitted
                    while any(pending[qq] for qq in range(k + 1)):
                        pop_filler()

                if quarters and "B" in stages:
                    # quarter-grained A: emit q0 eagerly, drip q1-q3 into B
                    # as PE filler; B(qmb k) flushes quarters <= k first.
                    xts0 = load_xts(0)
                    for it in quarter_items(xts0, 0, 0):
                        it()
                    xts1 = load_xts(1)
                    for qq in range(1, 4):
                        its = quarter_items(
                            xts0 if qq < 2 else xts1, qq // 2, qq % 2
                        )
                        pending[qq] = len(its)
                        filler.extend((qq, f) for f in its)
                else:
                    xts0 = load_xts(0)
                    for it in a_items(xts0, 0):
                        it()
                    xts1 = load_xts(1)
                    a1 = a_items(xts1, 1)
                    if a1_fill and "B" in stages:
                        filler.extend((None, f) for f in a1)
                    else:
                        for it in a1:
                            it()

                if "B" not in stages:
                    # sink so DCE keeps stage A
                    dma.dma_start(
                        out=out[0:128, 0:512],
                        in_=qkT[:, 0, 0:512],
                    )
                    dma.dma_start(
                        out=out[128:256, 0:260],
                        in_=v_aug[:, 0, :, :].rearrange("p h c -> p (h c)"),
                    )
                    return

                # ======== stage B+C: per query macro-block ========
                def make_c_item(st):
                    def emit():
                        ob = obp.tile([128, 1024], BF16)
                        for nt in range(2):
                            ps = ps_sc.tile([128, W], F32, tag="sc")
                            for dt_ in range(FV // 128):
                                nc.tensor.matmul(
                                    ps[:, 0:512],
                                    valuesT[:, dt_, st * 128:(st + 1) * 128],
                                    wo_sb[:, dt_, nt * 512:(nt + 1) * 512],
                                    start=(dt_ == 0),
                                    stop=(dt_ == FV // 128 - 1),
                                )
                            nc.vector.tensor_copy(
                                ob[:, nt * 512:(nt + 1) * 512], ps[:, 0:512]
                            )
                        dma.dma_start(
                            out=out[st * 128:(st + 1) * 128, :], in_=ob
                        )
                    return emit

                n_qmb_done = 0
                order = (0, 1, 2, 3) if quarters else qmb_order
                for qmb in order:
                    n_qmb_done += 1
                    if quarters:
                        need_quarter(qmb)
                    elif n_qmb_done == 3:
                        while filler:
                            pop_filler()
                    q0 = qmb * QMB
                    nkb = 4 * qmb + 4
                    nblk = (nkb + pairw - 1) // pairw
                    for w0 in range(0, HL, wave):
                        whs = list(range(w0, w0 + wave))
                        avs = {
                            h_: ps_av.tile([65, QMB], F32, tag="av",
                                           name=f"av{h_}")
                            for h_ in whs
                        }
                        avq = []

                        def emit_av(item):
                            h, mms = item
                            for kb, col0, avw, ex_t in mms:
                                nc.tensor.matmul(
                                    avs[h][0:65, col0:col0 + avw],
                                    v_aug[:, kb, h, :],
                                    ex_t,
                                    start=(kb == 0),
                                    stop=(kb == nkb - 1),
                                )

                        for blk in range(nblk):
                            kb0 = blk * pairw
                            scs = {}
                            # row-packed: both heads' score MMs emitted
                            # back-to-back; lhsT base partitions 0/64 ->
                            # concurrent row-group execution on the PE.
                            for h in whs:
                                scs[h] = ps_sc.tile(
                                    [128, W], F32, tag="sc", name=f"sc{h}"
                                )
                            for sub in range(pairw):
                                kb = kb0 + sub
                                j = kb - 4 * qmb
                                col0 = 128 * j if j >= 0 else 0
                                if pairw == 2 and kb0 == 4 * qmb and j == 1:
                                    # full-width so the j=0 pair's sc tile
                                    # has no PSUM hole (one full-tile exp)
                                    col0 = 0
                                cb = sub * 512 + col0
                                scw = 512 - col0
                                for h in whs:
                                    tk = 2 * (h // 2)
                                    pk = 64 * (h % 2)
                                    nc.tensor.matmul(
                                        scs[h][:, cb:cb + scw],
                                        qkT[pk:pk + 64, tk,
                                            kb * KB:(kb + 1) * KB],
                                        qkT[pk:pk + 64, tk + 1,
                                            q0 + col0:q0 + col0 + scw],
                                        start=True,
                                        stop=True,
                                        skip_group_check=True,
                                    )
                            for h in whs:
                                sc = scs[h]
                                ex = expp.tile([128, W], BF16)
                                diag = kb0 + pairw - 1 >= 4 * qmb
                                if tiny_exp:
                                    nc.scalar.activation(
                                        out=ex[:, 0:64],
                                        in_=sc[:, 0:64],
                                        func=mybir.ActivationFunctionType.Exp,
                                        scale=SCALE,
                                    )
                                elif diag and kb0 == 4 * qmb and pairw == 2:
                                    # pair straddles j=0,1: one full-tile exp
                                    # is cheaper than two partials (the +352
                                    # cycle instruction overhead dominates)
                                    nc.scalar.activation(
                                        out=ex,
                                        in_=sc,
                                        func=mybir.ActivationFunctionType.Exp,
                                        scale=SCALE,
                                    )
                                    for sub in range(pairw):
                                        j = kb0 + sub - 4 * qmb
                                        cb = sub * 512 + 128 * j
                                        nc.gpsimd.affine_select(
                                            out=ex[:, cb:cb + 128],
                                            in_=ex[:, cb:cb + 128],
                                            compare_op=mybir.AluOpType.is_ge,
                                            fill=0.0,
                                            base=0,
                                            pattern=[[1, 128]],
                                            channel_multiplier=-1,
                                        )
                                elif diag:
                                    for sub in range(pairw):
                                        j = kb0 + sub - 4 * qmb
                                        col0 = 128 * j if j >= 0 else 0
                                        cb = sub * 512 + col0
                                        nc.scalar.activation(
                                            out=ex[:, cb:sub * 512 + 512],
                                            in_=sc[:, cb:sub * 512 + 512],
                                            func=(mybir
                                                  .ActivationFunctionType.Exp),
                                            scale=SCALE,
                                        )
                                        if j >= 0:
                                            # zero ex where k > q in the
                                            # 128x128 diagonal chunk
                                            nc.gpsimd.affine_select(
                                                out=ex[:, cb:cb + 128],
                                                in_=ex[:, cb:cb + 128],
                                                compare_op=(mybir.AluOpType
                                                            .is_ge),
                                                fill=0.0,
                                                base=0,
                                                pattern=[[1, 128]],
                                                channel_multiplier=-1,
                                            )
                                else:
                                    nc.scalar.activation(
                                        out=ex,
                                        in_=sc,
                                        func=mybir.ActivationFunctionType.Exp,
                                        scale=SCALE,
                                    )
                                mms = []
                                for sub in range(pairw):
                                    kb = kb0 + sub
                                    j = kb - 4 * qmb
                                    col0 = 128 * j if j >= 0 else 0
                                    mms.append((
                                        kb, col0, QMB - col0,
                                        ex[:, sub * 512 + col0:
                                            sub * 512 + QMB],
                                    ))
                                avq.append((h, mms))
                            while len(avq) > wave * lag:
                                emit_av(avq.pop(0))
                            if filler:
                                pop_filler()
                        while avq:
                            emit_av(avq.pop(0))

                        # normalize: values = av[0:64] / av[64]
                        for h in whs:
                            av = avs[h]
                            rec = small.tile([1, QMB], BF16, tag="rec")
                            with nc.allow_low_precision(
                                reason="softmax denom feeds bf16 matmul"
                            ):
                                nc.vector.reciprocal(rec, av[64:65, :])
                            rb = small.tile([64, QMB], BF16, tag="rb")
                            nc.gpsimd.partition_broadcast(rb, rec)
                            dt_ = h // 2
                            pr = 64 * (h % 2)
                            nc.vector.tensor_mul(
                                valuesT[pr:pr + 64, dt_, q0:q0 + QMB],
                                av[0:64, :],
                                rb,
                            )
                    # ---- queue stage C for this qmb ----
                    if "C" not in stages:
                        dma.dma_start(
                            out=out[qmb * 128:(qmb + 1) * 128, 0:512],
                            in_=valuesT[:, 0, qmb * 512:qmb * 512 + 512],
                        )
                        continue
                    for sti in range(QMB // 128):
                        filler.append((None, make_c_item(qmb * 4 + sti)))
                while filler:
                    pop_filler()

            if repeat == 1:
                body(0)
            else:
                with tc.For_i(
                    0, repeat, 1,
                    hint_engines=(mybir.EngineType.PE,),
                ) as it:
                    body(it)
    nc.compile()
    return nc


def make_in_maps(x, W_qkv, b_qkv, W_out, b_out):
    """Host-side sharding: per-core input dict."""
    x = np.asarray(x, dtype=np.float32)
    W_qkv = np.asarray(W_qkv, dtype=np.float32)
    b_qkv = np.asarray(b_qkv, dtype=np.float32)
    W_out = np.asarray(W_out, dtype=np.float32)
    in_maps = []
    xT_by_b = [
        np.ascontiguousarray(x[b_].T).astype(BF16_NP) for b_ in range(B)
    ]
    for c in range(N_CORES):
        b_ = c // 4
        g = c % 4
        heads = [4 * g + i for i in range(HL)]
        # feature order: K(h0),K(h1),Q(h0),Q(h1),K(h2),K(h3),Q(h2),Q(h3)
        qk_cols = []
        for pair in range(2):
            h0, h1 = heads[2 * pair], heads[2 * pair + 1]
            for h_ in (h0, h1):
                base = h_ * 3 * HD + 1 * HD  # K
                qk_cols.extend(range(base, base + HD))
            for h_ in (h0, h1):
                base = h_ * 3 * HD + 0 * HD  # Q
                qk_cols.extend(range(base, base + HD))
        v_cols = []
        for h_ in heads:
            base = h_ * 3 * HD + 2 * HD  # V
            v_cols.extend(range(base, base + HD))
        qk_cols = np.array(qk_cols)
        v_cols = np.array(v_cols)
        bv_local = np.ascontiguousarray(b_qkv[v_cols], dtype=np.float32)
        in_maps.append({
            "xT": xT_by_b[b_],
            "wqk": np.ascontiguousarray(W_qkv[:, qk_cols]).astype(BF16_NP),
            "wv": np.ascontiguousarray(W_qkv[:, v_cols]).astype(BF16_NP),
            "wo": np.ascontiguousarray(
                W_out[g * FV:(g + 1) * FV, :]
            ).astype(BF16_NP),
            "bqk": np.ascontiguousarray(b_qkv[qk_cols], dtype=np.float32),
            "bv": np.tile(bv_local, 2),
        })
    return in_maps


_NC_CACHE = {}


def get_nc(repeat: int = 1):
    if repeat not in _NC_CACHE:
        _NC_CACHE[repeat] = build_kernel(repeat)
    return _NC_CACHE[repeat]


def kernel(x, W_qkv, b_qkv, W_out, b_out):
    in_maps = make_in_maps(x, W_qkv, b_qkv, W_out, b_out)
    nc = get_nc(1)
    res = run_bass_kernel_spmd(nc, in_maps, list(range(N_CORES)))
    b_out = np.asarray(b_out, dtype=np.float32)
    out = np.zeros((B, S, D), dtype=np.float32)
    for b_ in range(B):
        acc = np.zeros((S, D), dtype=np.float32)
        for g in range(4):
            acc += res.results[4 * b_ + g]["out"].astype(np.float32)
        out[b_] = acc + b_out[None, :]
    return out


# revision 28
# speedup vs baseline: 1.2065x; 1.0476x over previous
"""Multi-head causal attention (B=2, S=2048, D=1024, H=16) on 8 trn2 cores.

Sharding: core c -> (batch b = c//4, head-group g = c%4, 4 heads each).
Data-parallel over B, tensor-parallel over heads. Each core computes a
partial output projection [S, D]; the host sums the 4 partials per batch
and adds b_out.

v3 changes vs baseline (HW-validated on axon trn2, 8-core SPMD):
  - bf16 end to end (x, weights, qkT, v_aug, valuesT, ex, out partials);
    PSUM stays f32. Halves DMA traffic and most DVE copy traffic.
  - causal mask applied by zeroing ex diagonal triangles on GpSimd
    (affine_select) instead of PE mask matmuls.
  - qk bias folded into the PSUM->SBUF copy (tensor_scalar_add with a
    per-partition bias column); v bias folded into the v_aug copy
    (tensor_tensor add with a pre-broadcast bias tile).
  - xT DMA'd in 512-column chunks, stage A runs nt-major so the first
    matmul starts after ~0.5MB of DMA instead of ~4MB. xt pool is
    double-buffered so half-1 DMA overlaps half-0 compute.
  - quarters: only the first quarter of stage A runs ahead of B; the
    rest drips into B as PE filler (B is ACT/exp-bound on HW, so A's
    PE work hides under it). B(qmb k) flushes A quarters <= k first.
  - defnorm: softmax normalize is staged through one PSUM->SBUF copy
    so the av PSUM bank frees immediately; recip/broadcast/mul run
    off the critical path (-30us/iter on HW).

Device kernel per core (all matmuls bf16 -> f32 PSUM):
  A) qkT[f=512, s=2048] = (x @ Wqk)^T and v[s, f=256] = x @ Wv.
     qkT feature layout: [K(h0)|K(h1)] [Q(h0)|Q(h1)] [K(h2)|K(h3)] [Q(h2)|Q(h3)]
  B) per head h, per 512-wide query block qmb: causal flash attention in
     the scores-TRANSPOSED layout: sT[k,q] = K @ Q^T so that attn@V is
     lhsT=v_blk[s,hd+1] (ones col appended -> softmax denominators in
     row 64 of PSUM), rhs=expT[k,q]. No on-chip transposes anywhere.
  C) out_partial[s, 1024] = values^T.T @ W_out, DMA'd to DRAM (bf16).
"""
import math
import numpy as np
import ml_dtypes

import concourse.bass as bass
import concourse.mybir as mybir
import concourse.tile as tile
from concourse import bacc
from concourse.bass_utils import run_bass_kernel_spmd

N_CORES = 8
B, S, D = 2, 2048, 1024
H = 16                    # total heads
HL = 4                    # heads per core
HD = 64                   # head dim
FQK = 2 * HL * HD         # 512 local q+k features
FV = HL * HD              # 256 local v features
SCALE = 1.0 / math.sqrt(HD)

QMB = 512                 # query macro-block
KB = 128                  # key block
N_QMB = S // QMB          # 4
N_KB = S // KB            # 16

F32 = mybir.dt.float32
BF16 = mybir.dt.bfloat16
BF16_NP = ml_dtypes.bfloat16


def build_kernel(repeat: int = 1, pairw: int = 2, wave: int = 2,
                 sc_bufs: int = 3, av_bufs: int = 2, exp_bufs: int = 8,
                 lag: int = 1, qmb_order=(1, 2, 3, 0), a1_fill: bool = True,
                 stages: str = "ABC", tiny_exp: bool = False,
                 quarters: bool = True, defnorm: bool = True,
                 staggered: bool = False, fast_a: bool = False,
                 carry: bool = False):
    assert sc_bufs * pairw + av_bufs <= 8
    W = 512 * pairw
    nc = bacc.Bacc(
        "TRN2", target_bir_lowering=False, debug=False, num_devices=N_CORES
    )
    xT = nc.dram_tensor("xT", [D, S], BF16, kind="ExternalInput")
    wqk = nc.dram_tensor("wqk", [D, FQK], BF16, kind="ExternalInput")
    wv = nc.dram_tensor("wv", [D, FV], BF16, kind="ExternalInput")
    wo = nc.dram_tensor("wo", [FV, D], BF16, kind="ExternalInput")
    bqk = nc.dram_tensor("bqk", [FQK], F32, kind="ExternalInput")
    bv = nc.dram_tensor("bv", [2 * FV], F32, kind="ExternalInput")
    out = nc.dram_tensor("out", [S, D], BF16, kind="ExternalOutput")

    KT = D // 128  # 8 contraction tiles over D

    with tile.TileContext(nc) as tc:
        dma = nc.sync  # HWDGE: spreads transfers over HW queues
        with (
            tc.tile_pool(name="const", bufs=1) as const,
            tc.tile_pool(name="xt", bufs=2) as xtp,
            tc.tile_pool(name="big", bufs=1) as big,
            tc.tile_pool(name="exp", bufs=exp_bufs) as expp,
            tc.tile_pool(name="small", bufs=4) as small,
            tc.tile_pool(name="ob", bufs=3) as obp,
            tc.tile_pool(name="ps_sc", bufs=sc_bufs, space="PSUM") as ps_sc,
            tc.tile_pool(name="ps_av", bufs=av_bufs, space="PSUM") as ps_av,
        ):
            # ---- constants ----
            wqk_sb = const.tile([128, KT, FQK], BF16)
            wv_sb = const.tile([128, KT, FV], BF16)
            wo_sb = const.tile([128, FV // 128, D], BF16)
            dma.dma_start(
                out=wqk_sb, in_=wqk.rearrange("(kt p) f -> p kt f", p=128)
            )
            dma.dma_start(
                out=wv_sb, in_=wv.rearrange("(kt p) f -> p kt f", p=128)
            )
            dma.dma_start(
                out=wo_sb, in_=wo.rearrange("(dt p) f -> p dt f", p=128)
            )
            # qk bias as a per-partition column per f-tile: [128, ft]
            bqk_sb = const.tile([128, 4], F32)
            dma.dma_start(out=bqk_sb, in_=bqk.rearrange("(t p) -> p t", p=128))
            # v bias broadcast to all partitions: [128, 2*FV]
            bv_row = const.tile([1, 2 * FV], F32)
            dma.dma_start(out=bv_row, in_=bv.rearrange("(o f) -> o f", o=1))
            bv_bc = const.tile([128, 2 * FV], F32)
            nc.gpsimd.partition_broadcast(bv_bc, bv_row)

            # ---- persistent intermediates ----
            qkT = big.tile([128, 4, S], BF16)            # 4 f-tiles x S
            v_aug = big.tile([128, N_KB, HL, HD + 1], BF16)
            valuesT = big.tile([128, FV // 128, S], BF16)
            nc.vector.memset(v_aug[:, :, :, HD:HD + 1], 1.0)

            def body(_it):
                # ======== stage A: qkT and v_aug ========
                def load_xts(half):
                    s0 = half * (S // 2)
                    xts = []
                    for kt in range(KT):
                        xt_t = xtp.tile([128, S // 2], BF16, tag=f"xt{kt}")
                        for ch in range(2):
                            c0 = ch * 512
                            dma.dma_start(
                                out=xt_t[:, c0:c0 + 512],
                                in_=xT[kt * 128:(kt + 1) * 128,
                                       s0 + c0:s0 + c0 + 512],
                            )
                        xts.append(xt_t)
                    return xts

                def make_qk_item(xts, half, ft, nt):
                    s0 = half * (S // 2)

                    def emit():
                        c0 = nt * 512
                        ps = ps_sc.tile([128, W], F32, tag="sc")
                        for kt in range(KT):
                            nc.tensor.matmul(
                                ps[:, 0:512],
                                wqk_sb[:, kt, ft * 128:(ft + 1) * 128],
                                xts[kt][:, c0:c0 + 512],
                                start=(kt == 0),
                                stop=(kt == KT - 1),
                            )
                        nc.vector.tensor_scalar_add(
                            qkT[:, ft, s0 + c0:s0 + c0 + 512],
                            ps[:, 0:512],
                            bqk_sb[:, ft:ft + 1],
                        )
                    return emit

                def make_v_item(xts, half, stp):
                    def emit():
                        psv = ps_sc.tile([128, 512], F32, tag="sc")
                        for sub in range(2):
                            sti = stp * 2 + sub
                            c0 = sub * FV
                            for kt in range(KT):
                                nc.tensor.matmul(
                                    psv[:, c0:c0 + FV],
                                    xts[kt][:, sti * 128:(sti + 1) * 128],
                                    wv_sb[:, kt, :],
                                    start=(kt == 0),
                                    stop=(kt == KT - 1),
                                )
                        st0 = half * 8 + stp * 2
                        nc.vector.tensor_tensor(
                            v_aug[:, st0:st0 + 2, :, 0:HD],
                            psv.rearrange("s (t h c) -> s t h c", t=2, h=HL),
                            bv_bc.rearrange("s (t h c) -> s t h c", t=2, h=HL),
                            mybir.AluOpType.add,
                        )
                    return emit

                def a_items(xts, half):
                    # nt-major: items for columns 0:512 first, so compute
                    # starts after the first DMA chunk lands.
                    items = []
                    for nt in range(2):
                        for ft in range(4):
                            items.append(make_qk_item(xts, half, ft, nt))
                        items.append(make_v_item(xts, half, 2 * nt))
                        items.append(make_v_item(xts, half, 2 * nt + 1))
                    return items

                def quarter_items(xts, half, nt):
                    items = [
                        make_qk_item(xts, half, ft, nt) for ft in range(4)
                    ]
                    items.append(make_v_item(xts, half, 2 * nt))
                    items.append(make_v_item(xts, half, 2 * nt + 1))
                    return items

                # filler entries: (quarter_tag_or_None, fn)
                filler = []
                pending = [0, 0, 0, 0]  # un-emitted A items per quarter

                def pop_filler():
                    tag, fn = filler.pop(0)
                    fn()
                    if tag is not None:
                        pending[tag] -= 1

                def need_quarter(k):
                    # flush filler until A quarters <= k are all emitted
                    while any(pending[qq] for qq in range(k + 1)):
                        pop_filler()

                if quarters and "B" in stages:
                    # quarter-grained A: emit q0 eagerly, drip q1-q3 into B
                    # as PE filler; B(qmb k) flushes quarters <= k first.
                    xts0 = load_xts(0)
                    for it in quarter_items(xts0, 0, 0):
                        it()
                    xts1 = load_xts(1)
                    for qq in range(1, 4):
                        its = quarter_items(
                            xts0 if qq < 2 else xts1, qq // 2, qq % 2
                        )
                        pending[qq] = len(its)
                        filler.extend((qq, f) for f in its)
                else:
                    xts0 = load_xts(0)
                    for it in a_items(xts0, 0):
                        it()
                    xts1 = load_xts(1)
                    a1 = a_items(xts1, 1)
                    if a1_fill and "B" in stages:
                        filler.extend((None, f) for f in a1)
                    else:
                        for it in a1:
                            it()

                if "B" not in stages:
                    # sink so DCE keeps stage A
                    dma.dma_start(
                        out=out[0:128, 0:512],
                        in_=qkT[:, 0, 0:512],
                    )
                    dma.dma_start(
                        out=out[128:256, 0:260],
                        in_=v_aug[:, 0, :, :].rearrange("p h c -> p (h c)"),
                    )
                    return

                # ======== stage B+C: per query macro-block ========
                def make_c_item(st):
                    def emit():
                        ob = obp.tile([128, 1024], BF16)
                        for nt in range(2):
                            ps = ps_sc.tile([128, W], F32, tag="sc")
                            for dt_ in range(FV // 128):
                                nc.tensor.matmul(
                                    ps[:, 0:512],
                                    valuesT[:, dt_, st * 128:(st + 1) * 128],
                                    wo_sb[:, dt_, nt * 512:(nt + 1) * 512],
                                    start=(dt_ == 0),
                                    stop=(dt_ == FV // 128 - 1),
                                )
                            nc.vector.tensor_copy(
                                ob[:, nt * 512:(nt + 1) * 512], ps[:, 0:512]
                            )
                        dma.dma_start(
                            out=out[st * 128:(st + 1) * 128, :], in_=ob
                        )
                    return emit

                # carry: the final av MMs + normalize of wave w are
                # deferred into wave w+1's first block, so the next
                # wave's scores (and their exps) issue before the PE
                # round-trips on the last exps of this wave.
                pend = []

                def flush_pend():
                    while pend:
                        em, rem, nf = pend.pop(0)
                        for item in rem:
                            em(item)
                        nf()

                n_qmb_done = 0
                order = (0, 1, 2, 3) if quarters else qmb_order
                for qmb in order:
                    n_qmb_done += 1
                    if quarters:
                        need_quarter(qmb)
                    elif n_qmb_done == 3:
                        while filler:
                            pop_filler()
                    q0 = qmb * QMB
                    nkb = 4 * qmb + 4
                    nblk = (nkb + pairw - 1) // pairw
                    for w0 in range(0, HL, wave):
                        whs = list(range(w0, w0 + wave))
                        avs = {
                            h_: ps_av.tile([65, QMB], F32, tag="av",
                                           name=f"av{h_}")
                            for h_ in whs
                        }
                        avq = []

                        def emit_av(item, avs_=avs, nkb_=nkb):
                            h, mms = item
                            for kb, col0, avw, ex_t in mms:
                                nc.tensor.matmul(
                                    avs_[h][0:65, col0:col0 + avw],
                                    v_aug[:, kb, h, :],
                                    ex_t,
                                    start=(kb == 0),
                                    stop=(kb == nkb_ - 1),
                                )

                        for blk in range(nblk):
                            kb0 = blk * pairw
                            scs = {}
                            # row-packed: both heads' score MMs emitted
                            # back-to-back; lhsT base partitions 0/64 ->
                            # concurrent row-group execution on the PE.
                            for h in whs:
                                scs[h] = ps_sc.tile(
                                    [128, W], F32, tag="sc", name=f"sc{h}"
                                )
                            for sub in range(pairw):
                                kb = kb0 + sub
                                j = kb - 4 * qmb
                                col0 = 128 * j if j >= 0 else 0
                                if pairw == 2 and kb0 == 4 * qmb and j == 1:
                                    # full-width so the j=0 pair's sc tile
                                    # has no PSUM hole (one full-tile exp)
                                    col0 = 0
                                cb = sub * 512 + col0
                                scw = 512 - col0
                                for h in whs:
                                    tk = 2 * (h // 2)
                                    pk = 64 * (h % 2)
                                    nc.tensor.matmul(
                                        scs[h][:, cb:cb + scw],
                                        qkT[pk:pk + 64, tk,
                                            kb * KB:(kb + 1) * KB],
                                        qkT[pk:pk + 64, tk + 1,
                                            q0 + col0:q0 + col0 + scw],
                                        start=True,
                                        stop=True,
                                        skip_group_check=True,
                                    )
                            for h in whs:
                                sc = scs[h]
                                ex = expp.tile([128, W], BF16)
                                diag = kb0 + pairw - 1 >= 4 * qmb
                                if tiny_exp:
                                    nc.scalar.activation(
                                        out=ex[:, 0:64],
                                        in_=sc[:, 0:64],
                                        func=mybir.ActivationFunctionType.Exp,
                                        scale=SCALE,
                                    )
                                elif diag and kb0 == 4 * qmb and pairw == 2:
                                    # pair straddles j=0,1: one full-tile exp
                                    # is cheaper than two partials (the +352
                                    # cycle instruction overhead dominates)
                                    nc.scalar.activation(
                                        out=ex,
                                        in_=sc,
                                        func=mybir.ActivationFunctionType.Exp,
                                        scale=SCALE,
                                    )
                                    for sub in range(pairw):
                                        j = kb0 + sub - 4 * qmb
                                        cb = sub * 512 + 128 * j
                                        nc.gpsimd.affine_select(
                                            out=ex[:, cb:cb + 128],
                                            in_=ex[:, cb:cb + 128],
                                            compare_op=mybir.AluOpType.is_ge,
                                            fill=0.0,
                                            base=0,
                                            pattern=[[1, 128]],
                                            channel_multiplier=-1,
                                        )
                                elif diag:
                                    for sub in range(pairw):
                                        j = kb0 + sub - 4 * qmb
                                        col0 = 128 * j if j >= 0 else 0
                                        cb = sub * 512 + col0
                                        nc.scalar.activation(
                                            out=ex[:, cb:sub * 512 + 512],
                                            in_=sc[:, cb:sub * 512 + 512],
                                            func=(mybir
                                                  .ActivationFunctionType.Exp),
                                            scale=SCALE,
                                        )
                                        if j >= 0:
                                            # zero ex where k > q in the
                                            # 128x128 diagonal chunk
                                            nc.gpsimd.affine_select(
                                                out=ex[:, cb:cb + 128],
                                                in_=ex[:, cb:cb + 128],
                                                compare_op=(mybir.AluOpType
                                                            .is_ge),
                                                fill=0.0,
                                                base=0,
                                                pattern=[[1, 128]],
                                                channel_multiplier=-1,
                                            )
                                else:
                                    nc.scalar.activation(
                                        out=ex,
                                        in_=sc,
                                        func=mybir.ActivationFunctionType.Exp,
                                        scale=SCALE,
                                    )
                                mms = []
                                for sub in range(pairw):
                                    kb = kb0 + sub
                                    j = kb - 4 * qmb
                                    col0 = 128 * j if j >= 0 else 0
                                    mms.append((
                                        kb, col0, QMB - col0,
                                        ex[:, sub * 512 + col0:
                                            sub * 512 + QMB],
                                    ))
                                avq.append((h, mms))
                            if blk == 0:
                                flush_pend()
                            while len(avq) > wave * lag:
                                emit_av(avq.pop(0))
                            if filler:
                                pop_filler()
                            if (fast_a and filler
                                    and filler[0][0] is not None):
                                # drain A items at 2x so dependency
                                # flushes at qmb boundaries are short
                                pop_filler()

                        # normalize: values = av[0:64] / av[64].
                        # defnorm: one PSUM->SBUF copy frees the av bank
                        # for the next wave; recip/broadcast/mul run
                        # off-path from the SBUF staging tile.
                        def make_norm(whs_, avs_, q0_):
                            def nf():
                                for h in whs_:
                                    av = avs_[h]
                                    dt_ = h // 2
                                    pr = 64 * (h % 2)
                                    if defnorm:
                                        stg = small.tile(
                                            [65, QMB], F32, tag="stg"
                                        )
                                        nc.vector.tensor_copy(stg, av)
                                        src = stg
                                    else:
                                        src = av
                                    rec = small.tile([1, QMB], BF16,
                                                     tag="rec")
                                    with nc.allow_low_precision(
                                        reason="denom feeds bf16 matmul"
                                    ):
                                        nc.vector.reciprocal(
                                            rec, src[64:65, :]
                                        )
                                    rb = small.tile([64, QMB], BF16,
                                                    tag="rb")
                                    nc.gpsimd.partition_broadcast(rb, rec)
                                    nc.vector.tensor_mul(
                                        valuesT[pr:pr + 64, dt_,
                                                q0_:q0_ + QMB],
                                        src[0:64, :],
                                        rb,
                                    )
                            return nf

                        if carry:
                            pend.append(
                                (emit_av, list(avq), make_norm(whs, avs, q0))
                            )
                            avq.clear()
                        else:
                            while avq:
                                emit_av(avq.pop(0))
                            make_norm(whs, avs, q0)()
                    # ---- queue stage C for this qmb ----
                    if "C" not in stages:
                        dma.dma_start(
                            out=out[qmb * 128:(qmb + 1) * 128, 0:512],
                            in_=valuesT[:, 0, qmb * 512:qmb * 512 + 512],
                        )
                        continue
                    for sti in range(QMB // 128):
                        filler.append((None, make_c_item(qmb * 4 + sti)))
                flush_pend()
                while filler:
                    pop_filler()

            if repeat == 1:
                body(0)
            else:
                with tc.For_i(
                    0, repeat, 1,
                    hint_engines=(mybir.EngineType.PE,),
                    staggered_reset=staggered,
                ) as it:
                    body(it)
    nc.compile()
    return nc


def make_in_maps(x, W_qkv, b_qkv, W_out, b_out):
    """Host-side sharding: per-core input dict."""
    x = np.asarray(x, dtype=np.float32)
    W_qkv = np.asarray(W_qkv, dtype=np.float32)
    b_qkv = np.asarray(b_qkv, dtype=np.float32)
    W_out = np.asarray(W_out, dtype=np.float32)
    in_maps = []
    xT_by_b = [
        np.ascontiguousarray(x[b_].T).astype(BF16_NP) for b_ in range(B)
    ]
    for c in range(N_CORES):
        b_ = c // 4
        g = c % 4
        heads = [4 * g + i for i in range(HL)]
        # feature order: K(h0),K(h1),Q(h0),Q(h1),K(h2),K(h3),Q(h2),Q(h3)
        qk_cols = []
        for pair in range(2):
            h0, h1 = heads[2 * pair], heads[2 * pair + 1]
            for h_ in (h0, h1):
                base = h_ * 3 * HD + 1 * HD  # K
                qk_cols.extend(range(base, base + HD))
            for h_ in (h0, h1):
                base = h_ * 3 * HD + 0 * HD  # Q
                qk_cols.extend(range(base, base + HD))
        v_cols = []
        for h_ in heads:
            base = h_ * 3 * HD + 2 * HD  # V
            v_cols.extend(range(base, base + HD))
        qk_cols = np.array(qk_cols)
        v_cols = np.array(v_cols)
        bv_local = np.ascontiguousarray(b_qkv[v_cols], dtype=np.float32)
        in_maps.append({
            "xT": xT_by_b[b_],
            "wqk": np.ascontiguousarray(W_qkv[:, qk_cols]).astype(BF16_NP),
            "wv": np.ascontiguousarray(W_qkv[:, v_cols]).astype(BF16_NP),
            "wo": np.ascontiguousarray(
                W_out[g * FV:(g + 1) * FV, :]
            ).astype(BF16_NP),
            "bqk": np.ascontiguousarray(b_qkv[qk_cols], dtype=np.float32),
            "bv": np.tile(bv_local, 2),
        })
    return in_maps


_NC_CACHE = {}


def get_nc(repeat: int = 1):
    if repeat not in _NC_CACHE:
        _NC_CACHE[repeat] = build_kernel(repeat)
    return _NC_CACHE[repeat]


def kernel(x, W_qkv, b_qkv, W_out, b_out):
    in_maps = make_in_maps(x, W_qkv, b_qkv, W_out, b_out)
    nc = get_nc(1)
    res = run_bass_kernel_spmd(nc, in_maps, list(range(N_CORES)))
    b_out = np.asarray(b_out, dtype=np.float32)
    out = np.zeros((B, S, D), dtype=np.float32)
    for b_ in range(B):
        acc = np.zeros((S, D), dtype=np.float32)
        for g in range(4):
            acc += res.results[4 * b_ + g]["out"].astype(np.float32)
        out[b_] = acc + b_out[None, :]
    return out


# revision 34
# speedup vs baseline: 1.2532x; 1.0387x over previous
"""Multi-head causal attention (B=2, S=2048, D=1024, H=16) on 8 trn2 cores.

Sharding: core c -> (batch b = c//4, head-group g = c%4, 4 heads each).
Data-parallel over B, tensor-parallel over heads. Each core computes a
partial output projection [S, D]; the host sums the 4 partials per batch
and adds b_out.

v3 changes vs baseline (HW-validated on axon trn2, 8-core SPMD):
  - bf16 end to end (x, weights, qkT, v_aug, valuesT, ex, out partials);
    PSUM stays f32. Halves DMA traffic and most DVE copy traffic.
  - causal mask applied by zeroing ex diagonal triangles on GpSimd
    (affine_select) instead of PE mask matmuls.
  - qk bias folded into the PSUM->SBUF copy (tensor_scalar_add with a
    per-partition bias column); v bias folded into the v_aug copy
    (tensor_tensor add with a pre-broadcast bias tile).
  - xT DMA'd in 512-column chunks, stage A runs nt-major so the first
    matmul starts after ~0.5MB of DMA instead of ~4MB. xt pool is
    double-buffered so half-1 DMA overlaps half-0 compute.
  - quarters: only the first quarter of stage A runs ahead of B; the
    rest drips into B as PE filler (B is ACT/exp-bound on HW, so A's
    PE work hides under it). B(qmb k) flushes A quarters <= k first.
  - defnorm: softmax normalize is staged through one PSUM->SBUF copy
    so the av PSUM bank frees immediately; recip/broadcast/mul run
    off the critical path (-30us/iter on HW).

Device kernel per core (all matmuls bf16 -> f32 PSUM):
  A) qkT[f=512, s=2048] = (x @ Wqk)^T and v[s, f=256] = x @ Wv.
     qkT feature layout: [K(h0)|K(h1)] [Q(h0)|Q(h1)] [K(h2)|K(h3)] [Q(h2)|Q(h3)]
  B) per head h, per 512-wide query block qmb: causal flash attention in
     the scores-TRANSPOSED layout: sT[k,q] = K @ Q^T so that attn@V is
     lhsT=v_blk[s,hd+1] (ones col appended -> softmax denominators in
     row 64 of PSUM), rhs=expT[k,q]. No on-chip transposes anywhere.
  C) out_partial[s, 1024] = values^T.T @ W_out, DMA'd to DRAM (bf16).
"""
import math
import numpy as np
import ml_dtypes

import concourse.bass as bass
import concourse.mybir as mybir
import concourse.tile as tile
from concourse import bacc
from concourse.bass_utils import run_bass_kernel_spmd

N_CORES = 8
B, S, D = 2, 2048, 1024
H = 16                    # total heads
HL = 4                    # heads per core
HD = 64                   # head dim
FQK = 2 * HL * HD         # 512 local q+k features
FV = HL * HD              # 256 local v features
SCALE = 1.0 / math.sqrt(HD)

QMB = 512                 # query macro-block
KB = 128                  # key block
N_QMB = S // QMB          # 4
N_KB = S // KB            # 16

F32 = mybir.dt.float32
BF16 = mybir.dt.bfloat16
BF16_NP = ml_dtypes.bfloat16


def build_kernel(repeat: int = 1, pairw: int = 2, wave: int = 2,
                 sc_bufs: int = 3, av_bufs: int = 2, exp_bufs: int = 8,
                 lag: int = 1, qmb_order=(1, 2, 3, 0), a1_fill: bool = True,
                 stages: str = "ABC", tiny_exp: bool = False,
                 quarters: bool = True, defnorm: bool = True,
                 staggered: bool = False, fast_a: bool = False,
                 carry: bool = False, cslot1: bool = False,
                 qkpair: bool = False, unroll2: bool = False):
    assert sc_bufs * pairw + av_bufs <= 8
    W = 512 * pairw
    nc = bacc.Bacc(
        "TRN2", target_bir_lowering=False, debug=False, num_devices=N_CORES
    )
    xT = nc.dram_tensor("xT", [D, S], BF16, kind="ExternalInput")
    wqk = nc.dram_tensor("wqk", [D, FQK], BF16, kind="ExternalInput")
    wv = nc.dram_tensor("wv", [D, FV], BF16, kind="ExternalInput")
    wo = nc.dram_tensor("wo", [FV, D], BF16, kind="ExternalInput")
    bqk = nc.dram_tensor("bqk", [FQK], F32, kind="ExternalInput")
    bv = nc.dram_tensor("bv", [2 * FV], F32, kind="ExternalInput")
    out = nc.dram_tensor("out", [S, D], BF16, kind="ExternalOutput")

    KT = D // 128  # 8 contraction tiles over D

    with tile.TileContext(nc) as tc:
        dma = nc.sync  # HWDGE: spreads transfers over HW queues
        with (
            tc.tile_pool(name="const", bufs=1) as const,
            tc.tile_pool(name="xt", bufs=2) as xtp,
            tc.tile_pool(name="big", bufs=1) as big,
            tc.tile_pool(name="exp", bufs=exp_bufs) as expp,
            tc.tile_pool(name="small", bufs=4) as small,
            tc.tile_pool(name="ob", bufs=3) as obp,
            tc.tile_pool(name="ps_sc", bufs=sc_bufs, space="PSUM") as ps_sc,
            tc.tile_pool(name="ps_av", bufs=av_bufs, space="PSUM") as ps_av,
        ):
            # ---- constants ----
            wqk_sb = const.tile([128, KT, FQK], BF16)
            wv_sb = const.tile([128, KT, FV], BF16)
            wo_sb = const.tile([128, FV // 128, D], BF16)
            dma.dma_start(
                out=wqk_sb, in_=wqk.rearrange("(kt p) f -> p kt f", p=128)
            )
            dma.dma_start(
                out=wv_sb, in_=wv.rearrange("(kt p) f -> p kt f", p=128)
            )
            dma.dma_start(
                out=wo_sb, in_=wo.rearrange("(dt p) f -> p dt f", p=128)
            )
            # qk bias as a per-partition column per f-tile: [128, ft]
            bqk_sb = const.tile([128, 4], F32)
            dma.dma_start(out=bqk_sb, in_=bqk.rearrange("(t p) -> p t", p=128))
            # v bias broadcast to all partitions: [128, 2*FV]
            bv_row = const.tile([1, 2 * FV], F32)
            dma.dma_start(out=bv_row, in_=bv.rearrange("(o f) -> o f", o=1))
            bv_bc = const.tile([128, 2 * FV], F32)
            nc.gpsimd.partition_broadcast(bv_bc, bv_row)

            # ---- persistent intermediates ----
            qkT = big.tile([128, 4, S], BF16)            # 4 f-tiles x S
            v_aug = big.tile([128, N_KB, HL, HD + 1], BF16)
            valuesT = big.tile([128, FV // 128, S], BF16)
            nc.vector.memset(v_aug[:, :, :, HD:HD + 1], 1.0)

            def body(_it):
                # ======== stage A: qkT and v_aug ========
                def load_xts(half):
                    s0 = half * (S // 2)
                    xts = []
                    for kt in range(KT):
                        xt_t = xtp.tile([128, S // 2], BF16, tag=f"xt{kt}")
                        for ch in range(2):
                            c0 = ch * 512
                            dma.dma_start(
                                out=xt_t[:, c0:c0 + 512],
                                in_=xT[kt * 128:(kt + 1) * 128,
                                       s0 + c0:s0 + c0 + 512],
                            )
                        xts.append(xt_t)
                    return xts

                def make_qk_item(xts, half, ft, nt):
                    s0 = half * (S // 2)

                    def emit():
                        c0 = nt * 512
                        ps = ps_sc.tile([128, W], F32, tag="sc")
                        for kt in range(KT):
                            nc.tensor.matmul(
                                ps[:, 0:512],
                                wqk_sb[:, kt, ft * 128:(ft + 1) * 128],
                                xts[kt][:, c0:c0 + 512],
                                start=(kt == 0),
                                stop=(kt == KT - 1),
                            )
                        nc.vector.tensor_scalar_add(
                            qkT[:, ft, s0 + c0:s0 + c0 + 512],
                            ps[:, 0:512],
                            bqk_sb[:, ft:ft + 1],
                        )
                    return emit

                def make_qk_pair(xts, half, ft0, nt):
                    # two f-tiles through one 2-bank PSUM slot; the copy
                    # of half A overlaps the matmuls into half B (other
                    # bank), so filler items hold 1 slot instead of 2.
                    s0 = half * (S // 2)

                    def emit():
                        c0 = nt * 512
                        ps = ps_sc.tile([128, W], F32, tag="sc")
                        for i, ft in enumerate((ft0, ft0 + 1)):
                            pb = ps[:, i * 512:(i + 1) * 512]
                            for kt in range(KT):
                                nc.tensor.matmul(
                                    pb,
                                    wqk_sb[:, kt, ft * 128:(ft + 1) * 128],
                                    xts[kt][:, c0:c0 + 512],
                                    start=(kt == 0),
                                    stop=(kt == KT - 1),
                                )
                            nc.vector.tensor_scalar_add(
                                qkT[:, ft, s0 + c0:s0 + c0 + 512],
                                pb,
                                bqk_sb[:, ft:ft + 1],
                            )
                    return emit

                def make_v_item(xts, half, stp):
                    def emit():
                        psv = ps_sc.tile([128, 512], F32, tag="sc")
                        for sub in range(2):
                            sti = stp * 2 + sub
                            c0 = sub * FV
                            for kt in range(KT):
                                nc.tensor.matmul(
                                    psv[:, c0:c0 + FV],
                                    xts[kt][:, sti * 128:(sti + 1) * 128],
                                    wv_sb[:, kt, :],
                                    start=(kt == 0),
                                    stop=(kt == KT - 1),
                                )
                        st0 = half * 8 + stp * 2
                        nc.vector.tensor_tensor(
                            v_aug[:, st0:st0 + 2, :, 0:HD],
                            psv.rearrange("s (t h c) -> s t h c", t=2, h=HL),
                            bv_bc.rearrange("s (t h c) -> s t h c", t=2, h=HL),
                            mybir.AluOpType.add,
                        )
                    return emit

                def a_items(xts, half):
                    # nt-major: items for columns 0:512 first, so compute
                    # starts after the first DMA chunk lands.
                    items = []
                    for nt in range(2):
                        for ft in range(4):
                            items.append(make_qk_item(xts, half, ft, nt))
                        items.append(make_v_item(xts, half, 2 * nt))
                        items.append(make_v_item(xts, half, 2 * nt + 1))
                    return items

                def quarter_items(xts, half, nt):
                    if qkpair:
                        items = [
                            make_qk_pair(xts, half, 0, nt),
                            make_qk_pair(xts, half, 2, nt),
                        ]
                    else:
                        items = [
                            make_qk_item(xts, half, ft, nt)
                            for ft in range(4)
                        ]
                    items.append(make_v_item(xts, half, 2 * nt))
                    items.append(make_v_item(xts, half, 2 * nt + 1))
                    return items

                # filler entries: (quarter_tag_or_None, fn)
                filler = []
                pending = [0, 0, 0, 0]  # un-emitted A items per quarter

                def pop_filler():
                    tag, fn = filler.pop(0)
                    fn()
                    if tag is not None:
                        pending[tag] -= 1

                def need_quarter(k):
                    # flush filler until A quarters <= k are all emitted
                    while any(pending[qq] for qq in range(k + 1)):
                        pop_filler()

                if quarters and "B" in stages:
                    # quarter-grained A: emit q0 eagerly, drip q1-q3 into B
                    # as PE filler; B(qmb k) flushes quarters <= k first.
                    xts0 = load_xts(0)
                    for it in quarter_items(xts0, 0, 0):
                        it()
                    xts1 = load_xts(1)
                    for qq in range(1, 4):
                        its = quarter_items(
                            xts0 if qq < 2 else xts1, qq // 2, qq % 2
                        )
                        pending[qq] = len(its)
                        filler.extend((qq, f) for f in its)
                else:
                    xts0 = load_xts(0)
                    for it in a_items(xts0, 0):
                        it()
                    xts1 = load_xts(1)
                    a1 = a_items(xts1, 1)
                    if a1_fill and "B" in stages:
                        filler.extend((None, f) for f in a1)
                    else:
                        for it in a1:
                            it()

                if "B" not in stages:
                    # sink so DCE keeps stage A
                    dma.dma_start(
                        out=out[0:128, 0:512],
                        in_=qkT[:, 0, 0:512],
                    )
                    dma.dma_start(
                        out=out[128:256, 0:260],
                        in_=v_aug[:, 0, :, :].rearrange("p h c -> p (h c)"),
                    )
                    return

                # ======== stage B+C: per query macro-block ========
                def make_c_item(st):
                    def emit():
                        ob = obp.tile([128, 1024], BF16)
                        ps1 = ps_sc.tile([128, W], F32, tag="sc")
                        for nt in range(2):
                            if cslot1:
                                ps = ps1[:, nt * 512:(nt + 1) * 512]
                            elif nt == 0:
                                ps = ps1[:, 0:512]
                            else:
                                ps2 = ps_sc.tile(
                                    [128, W], F32, tag="sc", name="c_ps2"
                                )
                                ps = ps2[:, 0:512]
                            for dt_ in range(FV // 128):
                                nc.tensor.matmul(
                                    ps,
                                    valuesT[:, dt_, st * 128:(st + 1) * 128],
                                    wo_sb[:, dt_, nt * 512:(nt + 1) * 512],
                                    start=(dt_ == 0),
                                    stop=(dt_ == FV // 128 - 1),
                                )
                            nc.vector.tensor_copy(
                                ob[:, nt * 512:(nt + 1) * 512], ps
                            )
                        dma.dma_start(
                            out=out[st * 128:(st + 1) * 128, :], in_=ob
                        )
                    return emit

                # carry: the final av MMs + normalize of wave w are
                # deferred into wave w+1's first block, so the next
                # wave's scores (and their exps) issue before the PE
                # round-trips on the last exps of this wave.
                pend = []

                def flush_pend():
                    while pend:
                        em, rem, nf = pend.pop(0)
                        for item in rem:
                            em(item)
                        nf()

                n_qmb_done = 0
                order = (0, 1, 2, 3) if quarters else qmb_order
                for qmb in order:
                    n_qmb_done += 1
                    if quarters:
                        need_quarter(qmb)
                    elif n_qmb_done == 3:
                        while filler:
                            pop_filler()
                    q0 = qmb * QMB
                    nkb = 4 * qmb + 4
                    nblk = (nkb + pairw - 1) // pairw
                    for w0 in range(0, HL, wave):
                        whs = list(range(w0, w0 + wave))
                        avs = {
                            h_: ps_av.tile([65, QMB], F32, tag="av",
                                           name=f"av{h_}")
                            for h_ in whs
                        }
                        avq = []

                        def emit_av(item, avs_=avs, nkb_=nkb):
                            h, mms = item
                            for kb, col0, avw, ex_t in mms:
                                nc.tensor.matmul(
                                    avs_[h][0:65, col0:col0 + avw],
                                    v_aug[:, kb, h, :],
                                    ex_t,
                                    start=(kb == 0),
                                    stop=(kb == nkb_ - 1),
                                )

                        for blk in range(nblk):
                            kb0 = blk * pairw
                            scs = {}
                            # row-packed: both heads' score MMs emitted
                            # back-to-back; lhsT base partitions 0/64 ->
                            # concurrent row-group execution on the PE.
                            for h in whs:
                                scs[h] = ps_sc.tile(
                                    [128, W], F32, tag="sc", name=f"sc{h}"
                                )
                            for sub in range(pairw):
                                kb = kb0 + sub
                                j = kb - 4 * qmb
                                col0 = 128 * j if j >= 0 else 0
                                if pairw == 2 and kb0 == 4 * qmb and j == 1:
                                    # full-width so the j=0 pair's sc tile
                                    # has no PSUM hole (one full-tile exp)
                                    col0 = 0
                                cb = sub * 512 + col0
                                scw = 512 - col0
                                for h in whs:
                                    tk = 2 * (h // 2)
                                    pk = 64 * (h % 2)
                                    nc.tensor.matmul(
                                        scs[h][:, cb:cb + scw],
                                        qkT[pk:pk + 64, tk,
                                            kb * KB:(kb + 1) * KB],
                                        qkT[pk:pk + 64, tk + 1,
                                            q0 + col0:q0 + col0 + scw],
                                        start=True,
                                        stop=True,
                                        skip_group_check=True,
                                    )
                            for h in whs:
                                sc = scs[h]
                                ex = expp.tile([128, W], BF16)
                                diag = kb0 + pairw - 1 >= 4 * qmb
                                if tiny_exp:
                                    nc.scalar.activation(
                                        out=ex[:, 0:64],
                                        in_=sc[:, 0:64],
                                        func=mybir.ActivationFunctionType.Exp,
                                        scale=SCALE,
                                    )
                                elif diag and kb0 == 4 * qmb and pairw == 2:
                                    # pair straddles j=0,1: one full-tile exp
                                    # is cheaper than two partials (the +352
                                    # cycle instruction overhead dominates)
                                    nc.scalar.activation(
                                        out=ex,
                                        in_=sc,
                                        func=mybir.ActivationFunctionType.Exp,
                                        scale=SCALE,
                                    )
                                    for sub in range(pairw):
                                        j = kb0 + sub - 4 * qmb
                                        cb = sub * 512 + 128 * j
                                        nc.gpsimd.affine_select(
                                            out=ex[:, cb:cb + 128],
                                            in_=ex[:, cb:cb + 128],
                                            compare_op=mybir.AluOpType.is_ge,
                                            fill=0.0,
                                            base=0,
                                            pattern=[[1, 128]],
                                            channel_multiplier=-1,
                                        )
                                elif diag:
                                    for sub in range(pairw):
                                        j = kb0 + sub - 4 * qmb
                                        col0 = 128 * j if j >= 0 else 0
                                        cb = sub * 512 + col0
                                        nc.scalar.activation(
                                            out=ex[:, cb:sub * 512 + 512],
                                            in_=sc[:, cb:sub * 512 + 512],
                                            func=(mybir
                                                  .ActivationFunctionType.Exp),
                                            scale=SCALE,
                                        )
                                        if j >= 0:
                                            # zero ex where k > q in the
                                            # 128x128 diagonal chunk
                                            nc.gpsimd.affine_select(
                                                out=ex[:, cb:cb + 128],
                                                in_=ex[:, cb:cb + 128],
                                                compare_op=(mybir.AluOpType
                                                            .is_ge),
                                                fill=0.0,
                                                base=0,
                                                pattern=[[1, 128]],
                                                channel_multiplier=-1,
                                            )
                                else:
                                    nc.scalar.activation(
                                        out=ex,
                                        in_=sc,
                                        func=mybir.ActivationFunctionType.Exp,
                                        scale=SCALE,
                                    )
                                mms = []
                                for sub in range(pairw):
                                    kb = kb0 + sub
                                    j = kb - 4 * qmb
                                    col0 = 128 * j if j >= 0 else 0
                                    mms.append((
                                        kb, col0, QMB - col0,
                                        ex[:, sub * 512 + col0:
                                            sub * 512 + QMB],
                                    ))
                                avq.append((h, mms))
                            if blk == 0:
                                flush_pend()
                            while len(avq) > wave * lag:
                                emit_av(avq.pop(0))
                            if filler:
                                pop_filler()
                            if (fast_a and filler
                                    and filler[0][0] is not None):
                                # drain A items at 2x so dependency
                                # flushes at qmb boundaries are short
                                pop_filler()

                        # normalize: values = av[0:64] / av[64].
                        # defnorm: one PSUM->SBUF copy frees the av bank
                        # for the next wave; recip/broadcast/mul run
                        # off-path from the SBUF staging tile.
                        def make_norm(whs_, avs_, q0_):
                            def nf():
                                for h in whs_:
                                    av = avs_[h]
                                    dt_ = h // 2
                                    pr = 64 * (h % 2)
                                    if defnorm:
                                        stg = small.tile(
                                            [65, QMB], F32, tag="stg"
                                        )
                                        nc.vector.tensor_copy(stg, av)
                                        src = stg
                                    else:
                                        src = av
                                    rec = small.tile([1, QMB], BF16,
                                                     tag="rec")
                                    with nc.allow_low_precision(
                                        reason="denom feeds bf16 matmul"
                                    ):
                                        nc.vector.reciprocal(
                                            rec, src[64:65, :]
                                        )
                                    rb = small.tile([64, QMB], BF16,
                                                    tag="rb")
                                    nc.gpsimd.partition_broadcast(rb, rec)
                                    nc.vector.tensor_mul(
                                        valuesT[pr:pr + 64, dt_,
                                                q0_:q0_ + QMB],
                                        src[0:64, :],
                                        rb,
                                    )
                            return nf

                        if carry:
                            pend.append(
                                (emit_av, list(avq), make_norm(whs, avs, q0))
                            )
                            avq.clear()
                        else:
                            while avq:
                                emit_av(avq.pop(0))
                            make_norm(whs, avs, q0)()
                    # ---- queue stage C for this qmb ----
                    if "C" not in stages:
                        dma.dma_start(
                            out=out[qmb * 128:(qmb + 1) * 128, 0:512],
                            in_=valuesT[:, 0, qmb * 512:qmb * 512 + 512],
                        )
                        continue
                    for sti in range(QMB // 128):
                        filler.append((None, make_c_item(qmb * 4 + sti)))
                flush_pend()
                while filler:
                    pop_filler()

            if repeat == 1:
                body(0)
            elif unroll2 and repeat >= 2:
                # two bodies per loop iteration: halves the per-iteration
                # all-engine barrier/drain cost and lets body i+1's A
                # stage overlap body i's tail.
                n2 = repeat // 2
                with tc.For_i(
                    0, n2, 1,
                    hint_engines=(mybir.EngineType.PE,),
                    staggered_reset=staggered,
                ) as it:
                    body(it)
                    body(it)
                if repeat - 2 * n2:
                    body(0)
            else:
                with tc.For_i(
                    0, repeat, 1,
                    hint_engines=(mybir.EngineType.PE,),
                    staggered_reset=staggered,
                ) as it:
                    body(it)
    nc.compile()
    return nc


def make_in_maps(x, W_qkv, b_qkv, W_out, b_out):
    """Host-side sharding: per-core input dict."""
    x = np.asarray(x, dtype=np.float32)
    W_qkv = np.asarray(W_qkv, dtype=np.float32)
    b_qkv = np.asarray(b_qkv, dtype=np.float32)
    W_out = np.asarray(W_out, dtype=np.float32)
    in_maps = []
    xT_by_b = [
        np.ascontiguousarray(x[b_].T).astype(BF16_NP) for b_ in range(B)
    ]
    for c in range(N_CORES):
        b_ = c // 4
        g = c % 4
        heads = [4 * g + i for i in range(HL)]
        # feature order: K(h0),K(h1),Q(h0),Q(h1),K(h2),K(h3),Q(h2),Q(h3)
        qk_cols = []
        for pair in range(2):
            h0, h1 = heads[2 * pair], heads[2 * pair + 1]
            for h_ in (h0, h1):
                base = h_ * 3 * HD + 1 * HD  # K
                qk_cols.extend(range(base, base + HD))
            for h_ in (h0, h1):
                base = h_ * 3 * HD + 0 * HD  # Q
                qk_cols.extend(range(base, base + HD))
        v_cols = []
        for h_ in heads:
            base = h_ * 3 * HD + 2 * HD  # V
            v_cols.extend(range(base, base + HD))
        qk_cols = np.array(qk_cols)
        v_cols = np.array(v_cols)
        bv_local = np.ascontiguousarray(b_qkv[v_cols], dtype=np.float32)
        in_maps.append({
            "xT": xT_by_b[b_],
            "wqk": np.ascontiguousarray(W_qkv[:, qk_cols]).astype(BF16_NP),
            "wv": np.ascontiguousarray(W_qkv[:, v_cols]).astype(BF16_NP),
            "wo": np.ascontiguousarray(
                W_out[g * FV:(g + 1) * FV, :]
            ).astype(BF16_NP),
            "bqk": np.ascontiguousarray(b_qkv[qk_cols], dtype=np.float32),
            "bv": np.tile(bv_local, 2),
        })
    return in_maps


_NC_CACHE = {}


def get_nc(repeat: int = 1):
    if repeat not in _NC_CACHE:
        _NC_CACHE[repeat] = build_kernel(repeat)
    return _NC_CACHE[repeat]


def kernel(x, W_qkv, b_qkv, W_out, b_out):
    in_maps = make_in_maps(x, W_qkv, b_qkv, W_out, b_out)
    nc = get_nc(1)
    res = run_bass_kernel_spmd(nc, in_maps, list(range(N_CORES)))
    b_out = np.asarray(b_out, dtype=np.float32)
    out = np.zeros((B, S, D), dtype=np.float32)
    for b_ in range(B):
        acc = np.zeros((S, D), dtype=np.float32)
        for g in range(4):
            acc += res.results[4 * b_ + g]["out"].astype(np.float32)
        out[b_] = acc + b_out[None, :]
    return out
